# revision 41
# baseline (speedup 1.0000x reference)
"""Causal self-attention Bass/Tile kernel for Trainium2 (8 NeuronCores).

Problem: y = CausalSelfAttention(x) with
  B=8, T=1024, C=1024, H=16 heads, hs=64.
  qkv = x @ W_qkv + b_qkv;  per-head causal softmax(q k^T / sqrt(hs)) @ v;
  y = out @ W_proj + b_proj.

Sharding: pure data parallel - core i computes batch element i end-to-end.
No collectives.

Per-core plan (all matmuls bf16, fp32 PSUM accumulation):
  1. x is cast to bf16 on the host (no on-chip casts, half the HBM
     traffic of f32, and numerically BETTER: numpy rounds to nearest
     while the on-chip copy truncates).  Tiles stage through SBUF,
     striped over all three DMA queues in dependency-priority order;
     xT [C,T] is built with PE transposes (128x128 via identity matmuls)
     which also ramp the HAM clock before the first real matmul.
     (XBAR DMA transpose straight from DRAM returns nondeterministically
     wrong data, and SBUF-sourced XBAR moves only ~48GB/s of 256B
     packets while competing with the x loads for HBM - the PE does the
     whole 2MB in ~4us of otherwise-idle time.)
  2. qkT [2C,T] = (W_qk)^T x^T via matmuls (lhsT = W chunk, rhs = xT), with
     the 1/sqrt(hs) score scale pre-folded into W_q/b_q on the host.
     Weight chunks are pre-rearranged on the host to [m, p, co, r] so each
     chunk DMA reads 2KB/partition contiguously; chunks are prefetched two
     head-pairs ahead on rotating DMA rings.
  3. v [T,C] natural via matmuls (lhsT = xT chunk, rhs = W_v), stored bf16
     into v_pad [T, kb, h, 65] whose 65th column is ones (fused row-sum).
  4. Scores TRANSPOSED: S^T[k,q] tile = matmul(lhsT=kT chunk, rhs=qT), two
     heads packed onto PE row-groups (K=64 each) via tile_position.
     One wide exp per (head, key-block) on ACT straight out of a
     single-bank PSUM tile (no max subtraction needed; scores are O(1) by
     construction), bf16 out. Causal mask = multiplicative upper-tri mask
     on the diagonal 128x128 block only.
  5. PV: outT[h] [65, q] += matmul(lhsT=v_pad[:,kb,h,:], rhs=P^T tiles),
     emitted per 512-wide q-half so the projection can chase the last
     head-pair. Row 64 = softmax denominator s. Normalize: copy s row to
     SBUF, partition-broadcast on GPSIMD, reciprocal_approx_fast, multiply
     during the PSUM->SBUF copy of outT.
  6. proj: y [T,C] = matmul(lhsT=outT chunk, rhs=W_proj) + b_proj, emitted
     t-block-major: t-blocks 0-3 only need the last pair's first q-half,
     so the PE never idles (and never clock-gates) across the
     attention->projection seam. y DMAs alternate the two HWDGE rings.

Emission is software-pipelined across head-pairs (qkT pair j+1 and v
halves interleave with attention of pair j) so the PE never idles long
enough for the HAM clock-gate to re-throttle it to 1.2 GHz.
"""

import os
from contextlib import ExitStack

import numpy as np
import ml_dtypes

import concourse.bass as bass
import concourse.bacc as bacc
import concourse.mybir as mybir
import concourse.tile as tile
from concourse.bass_utils import run_bass_kernel_spmd

F32 = mybir.dt.float32
BF16 = mybir.dt.bfloat16

P = 128
B = 8
T = 1024
C = 1024
H = 16
HS = 64
TO = T // P   # 8 t-blocks
CO = C // P   # 8 c-chunks
NPAIR = H // 2  # 8 head pairs

# module-level knobs for test.py
TRACE = bool(int(os.environ.get("KERNEL_TRACE", "0")))
LAST_RESULTS = None  # BassKernelResults of last run


def build_nc():
    nc = bacc.Bacc("TRN2", target_bir_lowering=False, debug=False)

    x_d = nc.dram_tensor("x", [T, C], BF16, kind="ExternalInput").ap()
    # weight chunks pre-rearranged on the host for contiguous-per-partition
    # DMA: wqk[m, p, co, r] = W_qk[co*128+p, m*128+r], etc.
    wqk_d = nc.dram_tensor("wqk", [2 * CO, P, CO, P], BF16,
                           kind="ExternalInput").ap()
    wv_d = nc.dram_tensor("wv", [2, P, CO, 512], BF16,
                          kind="ExternalInput").ap()
    wproj_d = nc.dram_tensor("wproj", [2, P, CO, 512], BF16,
                             kind="ExternalInput").ap()
    bqk_d = nc.dram_tensor("bqk", [2 * C], F32, kind="ExternalInput").ap()
    bv_d = nc.dram_tensor("bv", [C], F32, kind="ExternalInput").ap()
    bproj_d = nc.dram_tensor("bproj", [C], F32, kind="ExternalInput").ap()
    mask_d = nc.dram_tensor("mask", [P, P], BF16, kind="ExternalInput").ap()
    ident_d = nc.dram_tensor("ident", [P, P], BF16, kind="ExternalInput").ap()
    y_d = nc.dram_tensor("y", [T, C], F32, kind="ExternalOutput").ap()

    with tile.TileContext(nc) as tc:
        _attn_body(tc, x_d, wqk_d, wv_d, wproj_d, bqk_d, bv_d, bproj_d,
                   mask_d, ident_d, y_d)
    nc.compile()
    return nc


def _attn_body(tc, x_d, wqk_d, wv_d, wproj_d, bqk_d, bv_d, bproj_d,
               mask_d, ident_d, y_d):
    nc = tc.nc
    with ExitStack() as ctx:
        # ---- pools that live the whole kernel ----
        consts = ctx.enter_context(tc.tile_pool(name="consts", bufs=1))
        big = ctx.enter_context(tc.tile_pool(name="big", bufs=1))
        ps_mm = ctx.enter_context(tc.tile_pool(name="ps_mm", bufs=3, space="PSUM"))

        wpp = tc.alloc_tile_pool(name="wprojp", bufs=2)
        wproj_sb = [None, None]
        xT_pool = tc.alloc_tile_pool(name="xT_pool", bufs=1)
        # xT[p, to, co, t128] = x[to*128+t128, co*128+p]; the inner [CO, P]
        # block is per-partition contiguous so one XBAR DMA transpose per
        # t-block writes it straight from DRAM.
        xT = xT_pool.tile([P, TO, CO, P], BF16, name="xT")  # 16KB/part

        # ---- resident activations ----
        qkT = big.tile([P, 2 * C // P, T], BF16, name="qkT")  # 32KB/part
        v_pad = big.tile([P, TO, H, HS + 1], BF16, name="v_pad")  # 16.6KB/part
        outT = big.tile([P, CO, T], BF16, name="outT")      # 16KB/part

        x_r = x_d.rearrange("(to p) c -> p to c", p=P)
        y_r = y_d.rearrange("(tb p) c -> p tb c", p=P)

        # attention-phase SBUF pools (released before the projection phase);
        # the attention PSUM pools are entered after phase T so the transpose
        # staging pool below can use those PSUM banks first.
        attn_ctx = ExitStack()
        ptp = attn_ctx.enter_context(tc.tile_pool(name="pt_pool", bufs=2))
        nrm = attn_ctx.enter_context(tc.tile_pool(name="nrm", bufs=2))
        wqkp = attn_ctx.enter_context(tc.tile_pool(name="wqk_pool", bufs=6))
        wvp = attn_ctx.enter_context(tc.tile_pool(name="wv_pool", bufs=2))

        # ============ Phase T: x load, PE transposes, weight prefetch ========
        # Startup is HBM-bound (~190GB/s/core while all 8 cores burst), so
        # DMA priority order is everything: the first matmul needs x t-blocks
        # 0-3 + the first weight chunk.  x is striped across all three DMA
        # queues; xT is built by PE transposes (128 cycles each on an
        # otherwise-idle engine - this also ramps the HAM clock early, and
        # beats the XBAR path, which moves only ~48GB/s of 256B packets while
        # competing with the x loads for HBM).
        w_tiles = {}

        def load_w(m, eng):
            w_m = wqkp.tile([P, CO, P], BF16, name=f"w_m{m}", tag="wqk")
            eng.dma_start(w_m, wqk_d[m])
            w_tiles[m] = w_m

        # identity (for PE transposes) + tiny consts lead the gpsimd ring,
        # then gpsimd helps carry x (one third of it) before the big wv load
        ident_sb = consts.tile([P, P], BF16, name="ident_sb")
        nc.gpsimd.dma_start(ident_sb, ident_d)
        xload = tc.alloc_tile_pool(name="xload", bufs=8)
        x_tiles = []
        rings = [nc.sync, nc.scalar]
        for to in range(4):
            x_t = xload.tile([P, C], BF16, name=f"x_t{to}", tag="xt")
            rings[to % 2].dma_start(x_t, x_r[:, to, :])
            x_tiles.append(x_t)
        load_w(0, nc.sync)
        load_w(NPAIR, nc.scalar)
        for to in range(4, TO):
            x_t = xload.tile([P, C], BF16, name=f"x_t{to}", tag="xt")
            rings[to % 2].dma_start(x_t, x_r[:, to, :])
            x_tiles.append(x_t)
        # warm the ACT exp table now - otherwise the first score exp pays a
        # 1.3us ACT_TABLE_LOAD right inside the critical scores(0) window
        warm = consts.tile([1, 1], F32, name="warm")
        nc.scalar.activation(out=warm, in_=ident_sb[0:1, 0:1],
                             func=mybir.ActivationFunctionType.Exp)
        bqk_sb = consts.tile([P, 2 * C // P], F32, name="bqk_sb")
        nc.gpsimd.dma_start(bqk_sb, bqk_d.rearrange("(m p) -> p m", p=P))
        mask_sb = consts.tile([P, P], BF16, name="mask_sb")
        nc.gpsimd.dma_start(mask_sb, mask_d)

        # Bias rows + broadcasts + the ones column of v_pad.
        rows_pool = tc.alloc_tile_pool(name="rows_pool", bufs=1)
        bv_row = rows_pool.tile([1, C], F32, name="bv_row")
        nc.gpsimd.dma_start(bv_row, bv_d[None, :])
        bproj_row = rows_pool.tile([1, C], F32, name="bproj_row")
        nc.gpsimd.dma_start(bproj_row, bproj_d[None, :])
        bv_bc = consts.tile([P, C], F32, name="bv_bc")
        bproj_bc = consts.tile([P, C], F32, name="bproj_bc")
        nc.gpsimd.partition_broadcast(bv_bc, bv_row)
        nc.gpsimd.partition_broadcast(bproj_bc, bproj_row)
        rows_pool.release()
        nc.vector.memset(v_pad[:, :, :, HS:HS + 1], 1.0)

        # First v half streams behind the consts on gpsimd.
        wv_sb = [None, None]

        def load_wv(n2, eng):
            wv_sb[n2] = wvp.tile([P, CO, 512], BF16, name=f"wv_sb{n2}",
                                 tag="wv")
            eng.dma_start(wv_sb[n2], wv_d[n2])

        load_wv(0, nc.gpsimd)

        # PE-transpose staging: 4 half-tiles in flight through PSUM
        ps_tr = tc.alloc_tile_pool(name="ps_tr", bufs=4, space="PSUM")

        def emit_xpose(to):
            """xT[:, to] = x_t^T via 8 PE transposes + 2 PSUM->SBUF copies."""
            x_t = x_tiles[to]
            for half in range(2):
                ps_t = ps_tr.tile([P, 512], BF16, name=f"tr{to}_{half}",
                                  tag="tr")
                for i in range(4):
                    co = half * 4 + i
                    nc.tensor.transpose(
                        ps_t[:, i * P:(i + 1) * P],
                        x_t[:, co * P:(co + 1) * P], ident_sb)
                if half == 0:
                    nc.vector.tensor_copy(
                        xT[:, to, half * 4:(half + 1) * 4, :], ps_t)
                else:
                    nc.scalar.copy(
                        xT[:, to, half * 4:(half + 1) * 4, :], ps_t)

        def emit_qkT(m, n2s=(0, 1)):
            """qkT rows m*128..m*128+127 (transposed): lhsT=W chunk, rhs=xT."""
            w_m = w_tiles[m]
            for n2 in n2s:
                ps = ps_mm.tile([P, 512], F32, name=f"qk_ps{m}_{n2}", tag="mm")
                for co in range(CO):
                    nc.tensor.matmul(
                        ps, w_m[:, co, :],
                        xT[:, n2 * 4:(n2 + 1) * 4, co, :],
                        start=(co == 0), stop=(co == CO - 1))
                nc.vector.tensor_scalar_add(
                    qkT[:, m, n2 * 512:(n2 + 1) * 512], ps, bqk_sb[:, m:m + 1])

        def emit_v_half(n2):
            """v columns n2*512..: heads 8*n2..8*n2+7, all t, bf16 + bias."""
            for tb in range(TO):
                ps = ps_mm.tile([P, 512], F32, name=f"v_ps{tb}_{n2}", tag="mm")
                for co in range(CO):
                    nc.tensor.matmul(
                        ps, xT[:, tb, co, :],
                        wv_sb[n2][:, co, :],
                        start=(co == 0), stop=(co == CO - 1))
                nc.vector.tensor_tensor(
                    out=v_pad[:, tb, n2 * 8:(n2 + 1) * 8, 0:HS],
                    in0=ps.rearrange("p (h d) -> p h d", d=HS),
                    in1=bv_bc[:, n2 * 512:(n2 + 1) * 512].rearrange(
                        "p (h d) -> p h d", d=HS),
                    op=mybir.AluOpType.add)

        def emit_scores(j):
            """S^T + exp + mask for both heads of pair j. Returns pt tiles."""
            pts = {}
            for hh in range(2):
                h = 2 * j + hh
                pb = hh * HS
                m_q, m_k = j, NPAIR + j
                for kb in range(TO):
                    w = T - kb * P
                    kT = qkT[pb:pb + HS, m_k, kb * P:(kb + 1) * P]
                    pt_kb = ptp.tile([P, w], BF16, name=f"pt{h}_{kb}",
                                     tag=f"pt{kb}")
                    # single-bank psum tiles + one exp per <=512 chunk:
                    # 4 chunks in flight decouple the PE from the scalar
                    # engine's exp throughput
                    off = 0
                    while off < w:
                        cw = min(512, w - off)
                        qs = kb * P + off
                        ps = ps_sc.tile([P, 512], F32,
                                        name=f"s{h}_{kb}_{off}", tag="sc")
                        nc.tensor.matmul(
                            ps[:, :cw], kT,
                            qkT[pb:pb + HS, m_q, qs:qs + cw],
                            start=True, stop=True, tile_position=(pb, 0))
                        nc.scalar.activation(
                            out=pt_kb[:, off:off + cw], in_=ps[:, :cw],
                            func=mybir.ActivationFunctionType.Exp)
                        off += cw
                    nc.vector.tensor_mul(pt_kb[:, 0:P], pt_kb[:, 0:P], mask_sb)
                    pts[(hh, kb)] = pt_kb
            return pts

        def emit_pv(j, qc, pts, hhs=(0, 1)):
            """PV + row-sum + normalization into outT, one 512-wide q-half."""
            for hh in hhs:
                h = 2 * j + hh
                pb = hh * HS
                ps_o = ps_pv.tile([HS + 1, 512], F32, name=f"o{h}_{qc}",
                                  tag="pv")
                kbs = [kb for kb in range(TO) if kb * P < (qc + 1) * 512]
                for i, kb in enumerate(kbs):
                    qlo = max(qc * 512, kb * P)
                    qhi = (qc + 1) * 512
                    nc.tensor.matmul(
                        ps_o[:, qlo - qc * 512:512],
                        v_pad[:, kb, h, :],
                        pts[(hh, kb)][:, qlo - kb * P:qhi - kb * P],
                        start=(i == 0), stop=(i == len(kbs) - 1))
                # normalization: fast reciprocal of the s row, GPSIMD
                # partition-broadcast, multiply during psum->sbuf copy
                srow = nrm.tile([1, 512], F32, name=f"sr{h}_{qc}",
                                tag="srow")
                nc.vector.tensor_copy(srow, ps_o[HS:HS + 1, :])
                rcp = nrm.tile([1, 512], F32, name=f"rcp{h}_{qc}",
                               tag="rcp")
                nc.vector.reciprocal_approx_fast(rcp, srow)
                bc = nrm.tile([HS, 512], F32, name=f"bc{h}_{qc}", tag="bc")
                nc.gpsimd.partition_broadcast(bc, rcp)
                nc.vector.tensor_mul(
                    outT[pb:pb + HS, j, qc * 512:(qc + 1) * 512],
                    ps_o[0:HS, :], bc)

        # ============ projection helpers ============
        # (yp is allocated in the emission sequence, after xload releases)
        y_ps = {}

        def proj_open(tb, n2, n_co):
            """Start the co-accumulation chain for one [128t, 512c] y tile."""
            ps = ps_mm.tile([P, 512], F32, name=f"y_ps{tb}_{n2}", tag="mm")
            for co in range(n_co):
                nc.tensor.matmul(
                    ps, outT[:, co, tb * P:(tb + 1) * P],
                    wproj_sb[n2][:, co, :],
                    start=(co == 0), stop=False)
            y_ps[(tb, n2)] = ps

        def proj_close(tb, n2):
            """Finish the chain (last co), add bias, store y."""
            ps = y_ps.pop((tb, n2))
            nc.tensor.matmul(
                ps, outT[:, CO - 1, tb * P:(tb + 1) * P],
                wproj_sb[n2][:, CO - 1, :],
                start=False, stop=True)
            y_sb = yp.tile([P, 512], F32, name=f"y_sb{tb}_{n2}", tag="y")
            if tb == TO - 1:
                # split the final stores so the kernel drains fast; keep
                # them off gpsimd, whose software queue signals drain late
                engs = ((nc.sync, nc.scalar) if n2 == 0
                        else (nc.scalar, nc.sync))
                nc.vector.tensor_add(
                    y_sb[:, 0:256], ps[:, 0:256],
                    bproj_bc[:, n2 * 512:n2 * 512 + 256])
                engs[0].dma_start(
                    y_r[:, tb, n2 * 512:n2 * 512 + 256], y_sb[:, 0:256])
                nc.vector.tensor_add(
                    y_sb[:, 256:512], ps[:, 256:512],
                    bproj_bc[:, n2 * 512 + 256:(n2 + 1) * 512])
                engs[1].dma_start(
                    y_r[:, tb, n2 * 512 + 256:(n2 + 1) * 512],
                    y_sb[:, 256:512])
            else:
                nc.vector.tensor_add(y_sb, ps,
                                     bproj_bc[:, n2 * 512:(n2 + 1) * 512])
                # 4MB of f32 output vs ~60GB/s per DMA queue: all three
                # queues are needed to drain y as fast as proj produces it
                # (gpsimd only for early tiles - its queue drains late)
                order = ((nc.sync, nc.scalar, nc.gpsimd) if tb < 5
                         else (nc.sync, nc.scalar, nc.scalar))
                eng = order[(2 * tb + n2) % 3]
                eng.dma_start(y_r[:, tb, n2 * 512:(n2 + 1) * 512], y_sb)

        # ============ pipelined main loop ============
        # transposes of t-blocks 0-3 unblock qkT m=0 n2=0; qkT m=8 n2=0
        # fills the PE while x t-blocks 4-7 are still landing.
        for to in range(4):
            emit_xpose(to)
        emit_qkT(0, (0,))
        emit_qkT(NPAIR, (0,))
        for to in range(4, TO):
            emit_xpose(to)
        ps_tr.release()
        xload.release()
        ps_sc = attn_ctx.enter_context(
            tc.tile_pool(name="ps_sc", bufs=3, space="PSUM"))
        ps_pv = attn_ctx.enter_context(
            tc.tile_pool(name="ps_pv", bufs=2, space="PSUM"))
        yp = tc.alloc_tile_pool(name="ypool", bufs=6)
        # pair-1/2 weight chunks chase the x tiles on the HWDGE rings
        load_w(1, nc.sync)
        load_w(NPAIR + 1, nc.scalar)
        emit_qkT(0, (1,))
        emit_qkT(NPAIR, (1,))
        pts = emit_scores(0)
        emit_v_half(0)
        # Software pipeline: iteration j runs qkT(j+1), scores(j+1), pv(j).
        # scores(j+1) leads pv(j+1) by a whole iteration so the scalar
        # engine's exp stream (~14.7us per pair, the 2nd-busiest resource)
        # stays off the PE's critical path; qkT(j+1) keeps the PE work DENSE
        # right through the tail so the HAM clock never gates down.
        for j in range(NPAIR):
            if j + 2 < NPAIR:
                load_w(j + 2, nc.sync)
                load_w(NPAIR + j + 2, nc.scalar)
            if j + 1 < NPAIR:
                emit_qkT(j + 1)
                emit_qkT(NPAIR + j + 1)
                pts_next = emit_scores(j + 1)
            if j == NPAIR - 1:
                # The last pair's exp stream (~14.7us of scalar) has nothing
                # behind it to hide under, so interleave everything that does
                # NOT depend on pair 7 between its PV pieces (emitted BEFORE
                # the stalling matmuls - the PE queue is in-order).
                proj_open(0, 0, CO - 1)
                proj_open(0, 1, CO - 1)
                proj_open(1, 0, CO - 1)
                emit_pv(j, 0, pts)
                emit_pv(j, 1, pts, hhs=(0,))
                proj_close(0, 0)
                proj_close(0, 1)
                proj_open(1, 1, CO - 1)
                emit_pv(j, 1, pts, hhs=(1,))
            else:
                emit_pv(j, 0, pts)
                emit_pv(j, 1, pts)
            if j + 1 < NPAIR:
                pts = pts_next
            if j == 0:
                load_wv(1, nc.gpsimd)
            if j == 1:
                emit_v_half(1)
            if j in (4, 5):
                n2 = j - 4
                wproj_sb[n2] = wpp.tile([P, CO, 512], BF16,
                                        name=f"wproj{n2}", tag="wproj")
                nc.gpsimd.dma_start(wproj_sb[n2], wproj_d[n2])

        # ============ Phase P: output projection ============
        # t-blocks 0-3 need only the last pair's first q-half (cols 0-511),
        # so their final-co matmuls overlap the last PV normalization; the
        # qc=1 normalization hides under tb2/tb3's chains.
        proj_close(1, 0)
        proj_close(1, 1)
        for tb in range(2, TO):
            for n2 in range(C // 512):
                proj_open(tb, n2, CO - 1)
                proj_close(tb, n2)
        yp.release()
        attn_ctx.close()
        xT_pool.release()
        wpp.release()


_NC_CACHE = None


def _get_nc():
    global _NC_CACHE
    if _NC_CACHE is None:
        _NC_CACHE = build_nc()
    return _NC_CACHE


def kernel(x, W_qkv, b_qkv, W_proj, b_proj):
    """Full-input entry point: shards batch across 8 cores, returns [B,T,C]."""
    global LAST_RESULTS
    x = np.asarray(x, dtype=np.float32)
    W_qkv = np.asarray(W_qkv, dtype=np.float32)
    b_qkv = np.asarray(b_qkv, dtype=np.float32)
    W_proj = np.asarray(W_proj, dtype=np.float32)
    b_proj = np.asarray(b_proj, dtype=np.float32)

    scale = 1.0 / np.sqrt(HS)
    wqk = W_qkv[:, :2 * C].copy()
    wqk[:, :C] *= scale
    bqk = b_qkv[:2 * C].copy()
    bqk[:C] *= scale
    # host-side chunking for contiguous-per-partition DMA:
    # wqk4[m, p, co, r] = wqk[co*128+p, m*128+r]
    wqk4 = np.ascontiguousarray(
        wqk.reshape(CO, P, 2 * CO, P).transpose(2, 1, 0, 3)
    ).astype(ml_dtypes.bfloat16)
    wv = np.ascontiguousarray(W_qkv[:, 2 * C:])
    wv4 = wv.reshape(CO, P, 2, 512).transpose(2, 1, 0, 3).astype(
        ml_dtypes.bfloat16)
    bv = np.ascontiguousarray(b_qkv[2 * C:])
    wp4 = W_proj.reshape(CO, P, 2, 512).transpose(2, 1, 0, 3).astype(
        ml_dtypes.bfloat16)
    # mask[k, q] = 1 where q >= k (valid, causal), else 0
    mask = np.triu(np.ones((P, P), dtype=np.float32)).astype(ml_dtypes.bfloat16)
    ident = np.eye(P, dtype=np.float32).astype(ml_dtypes.bfloat16)

    xb = x.astype(ml_dtypes.bfloat16)
    common = dict(wqk=wqk4, wv=wv4, wproj=wp4, bqk=bqk, bv=bv,
                  bproj=b_proj, mask=mask, ident=ident)
    in_maps = [dict(x=np.ascontiguousarray(xb[b]), **common) for b in range(B)]

    nc = _get_nc()
    res = run_bass_kernel_spmd(nc, in_maps, core_ids=list(range(B)),
                               trace=TRACE)
    LAST_RESULTS = res
    y = np.stack([res.results[b]["y"] for b in range(B)], axis=0)
    return y


# revision 42
# speedup vs baseline: 1.0051x; 1.0051x over previous
"""Causal self-attention Bass/Tile kernel for Trainium2 (8 NeuronCores).

Problem: y = CausalSelfAttention(x) with
  B=8, T=1024, C=1024, H=16 heads, hs=64.
  qkv = x @ W_qkv + b_qkv;  per-head causal softmax(q k^T / sqrt(hs)) @ v;
  y = out @ W_proj + b_proj.

Sharding: pure data parallel - core i computes batch element i end-to-end.
No collectives.

Per-core plan (all matmuls bf16, fp32 PSUM accumulation):
  1. x is cast to bf16 on the host (no on-chip casts, half the HBM
     traffic of f32, and numerically BETTER: numpy rounds to nearest
     while the on-chip copy truncates).  Tiles stage through SBUF,
     striped over all three DMA queues in dependency-priority order;
     xT [C,T] is built with PE transposes (128x128 via identity matmuls)
     which also ramp the HAM clock before the first real matmul.
     (XBAR DMA transpose straight from DRAM returns nondeterministically
     wrong data, and SBUF-sourced XBAR moves only ~48GB/s of 256B
     packets while competing with the x loads for HBM - the PE does the
     whole 2MB in ~4us of otherwise-idle time.)
  2. qkT [2C,T] = (W_qk)^T x^T via matmuls (lhsT = W chunk, rhs = xT), with
     the 1/sqrt(hs) score scale pre-folded into W_q/b_q on the host.
     Weight chunks are pre-rearranged on the host to [m, p, co, r] so each
     chunk DMA reads 2KB/partition contiguously; chunks are prefetched two
     head-pairs ahead on rotating DMA rings.
  3. v [T,C] natural via matmuls (lhsT = xT chunk, rhs = W_v), stored bf16
     into v_pad [T, kb, h, 65] whose 65th column is ones (fused row-sum).
  4. Scores TRANSPOSED: S^T[k,q] tile = matmul(lhsT=kT chunk, rhs=qT), two
     heads packed onto PE row-groups (K=64 each) via tile_position.
     One wide exp per (head, key-block) on ACT straight out of a
     single-bank PSUM tile (no max subtraction needed; scores are O(1) by
     construction), bf16 out. Causal mask = multiplicative upper-tri mask
     on the diagonal 128x128 block only.
  5. PV: outT[h] [65, q] += matmul(lhsT=v_pad[:,kb,h,:], rhs=P^T tiles),
     emitted per 512-wide q-half so the projection can chase the last
     head-pair. Row 64 = softmax denominator s. Normalize: copy s row to
     SBUF, partition-broadcast on GPSIMD, reciprocal_approx_fast, multiply
     during the PSUM->SBUF copy of outT.
  6. proj: y [T,C] = matmul(lhsT=outT chunk, rhs=W_proj) + b_proj, emitted
     t-block-major: t-blocks 0-3 only need the last pair's first q-half,
     so the PE never idles (and never clock-gates) across the
     attention->projection seam. y DMAs alternate the two HWDGE rings.

Emission is software-pipelined across head-pairs (qkT pair j+1 and v
halves interleave with attention of pair j) so the PE never idles long
enough for the HAM clock-gate to re-throttle it to 1.2 GHz.
"""

import os
from contextlib import ExitStack

import numpy as np
import ml_dtypes

import concourse.bass as bass
import concourse.bacc as bacc
import concourse.mybir as mybir
import concourse.tile as tile
from concourse.bass_utils import run_bass_kernel_spmd

F32 = mybir.dt.float32
BF16 = mybir.dt.bfloat16

P = 128
B = 8
T = 1024
C = 1024
H = 16
HS = 64
TO = T // P   # 8 t-blocks
CO = C // P   # 8 c-chunks
NPAIR = H // 2  # 8 head pairs

# module-level knobs for test.py
TRACE = bool(int(os.environ.get("KERNEL_TRACE", "0")))
LAST_RESULTS = None  # BassKernelResults of last run


def build_nc():
    nc = bacc.Bacc("TRN2", target_bir_lowering=False, debug=False)

    x_d = nc.dram_tensor("x", [T, C], BF16, kind="ExternalInput").ap()
    # weight chunks pre-rearranged on the host for contiguous-per-partition
    # DMA: wqk[m, p, co, r] = W_qk[co*128+p, m*128+r], etc.
    wqk_d = nc.dram_tensor("wqk", [2 * CO, P, CO, P], BF16,
                           kind="ExternalInput").ap()
    wv_d = nc.dram_tensor("wv", [2, P, CO, 512], BF16,
                          kind="ExternalInput").ap()
    wproj_d = nc.dram_tensor("wproj", [2, P, CO, 512], BF16,
                             kind="ExternalInput").ap()
    bqk_d = nc.dram_tensor("bqk", [2 * C], F32, kind="ExternalInput").ap()
    bv_d = nc.dram_tensor("bv", [C], F32, kind="ExternalInput").ap()
    bproj_d = nc.dram_tensor("bproj", [C], F32, kind="ExternalInput").ap()
    mask_d = nc.dram_tensor("mask", [P, P], BF16, kind="ExternalInput").ap()
    ident_d = nc.dram_tensor("ident", [P, P], BF16, kind="ExternalInput").ap()
    y_d = nc.dram_tensor("y", [T, C], F32, kind="ExternalOutput").ap()

    with tile.TileContext(nc) as tc:
        _attn_body(tc, x_d, wqk_d, wv_d, wproj_d, bqk_d, bv_d, bproj_d,
                   mask_d, ident_d, y_d)
    nc.compile()
    return nc


def _attn_body(tc, x_d, wqk_d, wv_d, wproj_d, bqk_d, bv_d, bproj_d,
               mask_d, ident_d, y_d):
    nc = tc.nc
    with ExitStack() as ctx:
        # ---- pools that live the whole kernel ----
        consts = ctx.enter_context(tc.tile_pool(name="consts", bufs=1))
        big = ctx.enter_context(tc.tile_pool(name="big", bufs=1))
        ps_mm = ctx.enter_context(tc.tile_pool(name="ps_mm", bufs=3, space="PSUM"))

        wpp = tc.alloc_tile_pool(name="wprojp", bufs=2)
        wproj_sb = [None, None]
        xT_pool = tc.alloc_tile_pool(name="xT_pool", bufs=1)
        # xT[p, to, co, t128] = x[to*128+t128, co*128+p]; the inner [CO, P]
        # block is per-partition contiguous so one XBAR DMA transpose per
        # t-block writes it straight from DRAM.
        xT = xT_pool.tile([P, TO, CO, P], BF16, name="xT")  # 16KB/part

        # ---- resident activations ----
        qkT = big.tile([P, 2 * C // P, T], BF16, name="qkT")  # 32KB/part
        v_pad = big.tile([P, TO, H, HS + 1], BF16, name="v_pad")  # 16.6KB/part
        outT = big.tile([P, CO, T], BF16, name="outT")      # 16KB/part

        x_r = x_d.rearrange("(to p) c -> p to c", p=P)
        y_r = y_d.rearrange("(tb p) c -> p tb c", p=P)

        # attention-phase SBUF pools (released before the projection phase);
        # the attention PSUM pools are entered after phase T so the transpose
        # staging pool below can use those PSUM banks first.
        attn_ctx = ExitStack()
        ptp = attn_ctx.enter_context(tc.tile_pool(name="pt_pool", bufs=2))
        nrm = attn_ctx.enter_context(tc.tile_pool(name="nrm", bufs=2))
        wqkp = attn_ctx.enter_context(tc.tile_pool(name="wqk_pool", bufs=6))
        wvp = attn_ctx.enter_context(tc.tile_pool(name="wv_pool", bufs=2))

        # ============ Phase T: x load, PE transposes, weight prefetch ========
        # Startup is HBM-bound (~190GB/s/core while all 8 cores burst), so
        # DMA priority order is everything: the first matmul needs x t-blocks
        # 0-3 + the first weight chunk.  x is striped across all three DMA
        # queues; xT is built by PE transposes (128 cycles each on an
        # otherwise-idle engine - this also ramps the HAM clock early, and
        # beats the XBAR path, which moves only ~48GB/s of 256B packets while
        # competing with the x loads for HBM).
        w_tiles = {}

        def load_w(m, eng):
            w_m = wqkp.tile([P, CO, P], BF16, name=f"w_m{m}", tag="wqk")
            eng.dma_start(w_m, wqk_d[m])
            w_tiles[m] = w_m

        # identity (for PE transposes) + tiny consts lead the gpsimd ring,
        # then gpsimd helps carry x (one third of it) before the big wv load
        ident_sb = consts.tile([P, P], BF16, name="ident_sb")
        nc.gpsimd.dma_start(ident_sb, ident_d)
        xload = tc.alloc_tile_pool(name="xload", bufs=8)
        x_tiles = []
        rings = [nc.sync, nc.scalar]
        for to in range(4):
            x_t = xload.tile([P, C], BF16, name=f"x_t{to}", tag="xt")
            rings[to % 2].dma_start(x_t, x_r[:, to, :])
            x_tiles.append(x_t)
        load_w(0, nc.sync)
        load_w(NPAIR, nc.scalar)
        for to in range(4, TO):
            x_t = xload.tile([P, C], BF16, name=f"x_t{to}", tag="xt")
            rings[to % 2].dma_start(x_t, x_r[:, to, :])
            x_tiles.append(x_t)
        # warm the ACT exp table now - otherwise the first score exp pays a
        # 1.3us ACT_TABLE_LOAD right inside the critical scores(0) window
        warm = consts.tile([1, 1], F32, name="warm")
        nc.scalar.activation(out=warm, in_=ident_sb[0:1, 0:1],
                             func=mybir.ActivationFunctionType.Exp)
        bqk_sb = consts.tile([P, 2 * C // P], F32, name="bqk_sb")
        nc.gpsimd.dma_start(bqk_sb, bqk_d.rearrange("(m p) -> p m", p=P))
        mask_sb = consts.tile([P, P], BF16, name="mask_sb")
        nc.gpsimd.dma_start(mask_sb, mask_d)

        # Bias rows + broadcasts + the ones column of v_pad.
        rows_pool = tc.alloc_tile_pool(name="rows_pool", bufs=1)
        bv_row = rows_pool.tile([1, C], F32, name="bv_row")
        nc.gpsimd.dma_start(bv_row, bv_d[None, :])
        bproj_row = rows_pool.tile([1, C], F32, name="bproj_row")
        nc.gpsimd.dma_start(bproj_row, bproj_d[None, :])
        bv_bc = consts.tile([P, C], F32, name="bv_bc")
        bproj_bc = consts.tile([P, C], F32, name="bproj_bc")
        nc.gpsimd.partition_broadcast(bv_bc, bv_row)
        nc.gpsimd.partition_broadcast(bproj_bc, bproj_row)
        rows_pool.release()
        nc.vector.memset(v_pad[:, :, :, HS:HS + 1], 1.0)

        # First v half streams behind the consts on gpsimd.
        wv_sb = [None, None]

        def load_wv(n2, eng):
            wv_sb[n2] = wvp.tile([P, CO, 512], BF16, name=f"wv_sb{n2}",
                                 tag="wv")
            eng.dma_start(wv_sb[n2], wv_d[n2])

        load_wv(0, nc.gpsimd)

        # PE-transpose staging: 4 half-tiles in flight through PSUM
        ps_tr = tc.alloc_tile_pool(name="ps_tr", bufs=4, space="PSUM")

        # HAM clock warmup: the PE ramps 0.65->1.2->2.4GHz only after ~3us of
        # sustained activity, so the prelude (transposes + first qkT pairs)
        # otherwise pays a ~2x clock tax.  Dummy transposes of a memset
        # scratch tile depend on nothing but the memset - they start the
        # ramp ~4us before x t-block 0 even lands, and finish before the
        # first real transpose's input is available, so they delay nothing.
        scratch = consts.tile([P, P], BF16, name="scratch")
        nc.vector.memset(scratch, 0.0)
        for wu in range(4):
            ps_t = ps_tr.tile([P, 512], BF16, name=f"warm{wu}", tag="tr")
            for i in range(4):
                nc.tensor.transpose(ps_t[:, i * P:(i + 1) * P],
                                    scratch, scratch)

        def emit_xpose(to):
            """xT[:, to] = x_t^T via 8 PE transposes + 2 PSUM->SBUF copies."""
            x_t = x_tiles[to]
            for half in range(2):
                ps_t = ps_tr.tile([P, 512], BF16, name=f"tr{to}_{half}",
                                  tag="tr")
                for i in range(4):
                    co = half * 4 + i
                    nc.tensor.transpose(
                        ps_t[:, i * P:(i + 1) * P],
                        x_t[:, co * P:(co + 1) * P], ident_sb)
                if half == 0:
                    nc.vector.tensor_copy(
                        xT[:, to, half * 4:(half + 1) * 4, :], ps_t)
                else:
                    nc.scalar.copy(
                        xT[:, to, half * 4:(half + 1) * 4, :], ps_t)

        def emit_qkT(m, n2s=(0, 1)):
            """qkT rows m*128..m*128+127 (transposed): lhsT=W chunk, rhs=xT."""
            w_m = w_tiles[m]
            for n2 in n2s:
                ps = ps_mm.tile([P, 512], F32, name=f"qk_ps{m}_{n2}", tag="mm")
                for co in range(CO):
                    nc.tensor.matmul(
                        ps, w_m[:, co, :],
                        xT[:, n2 * 4:(n2 + 1) * 4, co, :],
                        start=(co == 0), stop=(co == CO - 1))
                nc.vector.tensor_scalar_add(
                    qkT[:, m, n2 * 512:(n2 + 1) * 512], ps, bqk_sb[:, m:m + 1])

        def emit_v_half(n2):
            """v columns n2*512..: heads 8*n2..8*n2+7, all t, bf16 + bias."""
            for tb in range(TO):
                ps = ps_mm.tile([P, 512], F32, name=f"v_ps{tb}_{n2}", tag="mm")
                for co in range(CO):
                    nc.tensor.matmul(
                        ps, xT[:, tb, co, :],
                        wv_sb[n2][:, co, :],
                        start=(co == 0), stop=(co == CO - 1))
                nc.vector.tensor_tensor(
                    out=v_pad[:, tb, n2 * 8:(n2 + 1) * 8, 0:HS],
                    in0=ps.rearrange("p (h d) -> p h d", d=HS),
                    in1=bv_bc[:, n2 * 512:(n2 + 1) * 512].rearrange(
                        "p (h d) -> p h d", d=HS),
                    op=mybir.AluOpType.add)

        def emit_scores(j):
            """S^T + exp + mask for both heads of pair j. Returns pt tiles."""
            pts = {}
            for hh in range(2):
                h = 2 * j + hh
                pb = hh * HS
                m_q, m_k = j, NPAIR + j
                for kb in range(TO):
                    w = T - kb * P
                    kT = qkT[pb:pb + HS, m_k, kb * P:(kb + 1) * P]
                    pt_kb = ptp.tile([P, w], BF16, name=f"pt{h}_{kb}",
                                     tag=f"pt{kb}")
                    # single-bank psum tiles + one exp per <=512 chunk:
                    # 4 chunks in flight decouple the PE from the scalar
                    # engine's exp throughput
                    off = 0
                    while off < w:
                        cw = min(512, w - off)
                        qs = kb * P + off
                        ps = ps_sc.tile([P, 512], F32,
                                        name=f"s{h}_{kb}_{off}", tag="sc")
                        nc.tensor.matmul(
                            ps[:, :cw], kT,
                            qkT[pb:pb + HS, m_q, qs:qs + cw],
                            start=True, stop=True, tile_position=(pb, 0))
                        nc.scalar.activation(
                            out=pt_kb[:, off:off + cw], in_=ps[:, :cw],
                            func=mybir.ActivationFunctionType.Exp)
                        off += cw
                    nc.vector.tensor_mul(pt_kb[:, 0:P], pt_kb[:, 0:P], mask_sb)
                    pts[(hh, kb)] = pt_kb
            return pts

        def emit_pv(j, qc, pts, hhs=(0, 1)):
            """PV + row-sum + normalization into outT, one 512-wide q-half."""
            for hh in hhs:
                h = 2 * j + hh
                pb = hh * HS
                ps_o = ps_pv.tile([HS + 1, 512], F32, name=f"o{h}_{qc}",
                                  tag="pv")
                kbs = [kb for kb in range(TO) if kb * P < (qc + 1) * 512]
                for i, kb in enumerate(kbs):
                    qlo = max(qc * 512, kb * P)
                    qhi = (qc + 1) * 512
                    nc.tensor.matmul(
                        ps_o[:, qlo - qc * 512:512],
                        v_pad[:, kb, h, :],
                        pts[(hh, kb)][:, qlo - kb * P:qhi - kb * P],
                        start=(i == 0), stop=(i == len(kbs) - 1))
                # normalization: fast reciprocal of the s row, GPSIMD
                # partition-broadcast, multiply during psum->sbuf copy
                srow = nrm.tile([1, 512], F32, name=f"sr{h}_{qc}",
                                tag="srow")
                nc.vector.tensor_copy(srow, ps_o[HS:HS + 1, :])
                rcp = nrm.tile([1, 512], F32, name=f"rcp{h}_{qc}",
                               tag="rcp")
                nc.vector.reciprocal_approx_fast(rcp, srow)
                bc = nrm.tile([HS, 512], F32, name=f"bc{h}_{qc}", tag="bc")
                nc.gpsimd.partition_broadcast(bc, rcp)
                nc.vector.tensor_mul(
                    outT[pb:pb + HS, j, qc * 512:(qc + 1) * 512],
                    ps_o[0:HS, :], bc)

        # ============ projection helpers ============
        # (yp is allocated in the emission sequence, after xload releases)
        y_ps = {}

        def proj_open(tb, n2, n_co):
            """Start the co-accumulation chain for one [128t, 512c] y tile."""
            ps = ps_mm.tile([P, 512], F32, name=f"y_ps{tb}_{n2}", tag="mm")
            for co in range(n_co):
                nc.tensor.matmul(
                    ps, outT[:, co, tb * P:(tb + 1) * P],
                    wproj_sb[n2][:, co, :],
                    start=(co == 0), stop=False)
            y_ps[(tb, n2)] = ps

        def proj_close(tb, n2):
            """Finish the chain (last co), add bias, store y."""
            ps = y_ps.pop((tb, n2))
            nc.tensor.matmul(
                ps, outT[:, CO - 1, tb * P:(tb + 1) * P],
                wproj_sb[n2][:, CO - 1, :],
                start=False, stop=True)
            y_sb = yp.tile([P, 512], F32, name=f"y_sb{tb}_{n2}", tag="y")
            if tb == TO - 1:
                # split the final stores so the kernel drains fast; keep
                # them off gpsimd, whose software queue signals drain late
                engs = ((nc.sync, nc.scalar) if n2 == 0
                        else (nc.scalar, nc.sync))
                nc.vector.tensor_add(
                    y_sb[:, 0:256], ps[:, 0:256],
                    bproj_bc[:, n2 * 512:n2 * 512 + 256])
                engs[0].dma_start(
                    y_r[:, tb, n2 * 512:n2 * 512 + 256], y_sb[:, 0:256])
                nc.vector.tensor_add(
                    y_sb[:, 256:512], ps[:, 256:512],
                    bproj_bc[:, n2 * 512 + 256:(n2 + 1) * 512])
                engs[1].dma_start(
                    y_r[:, tb, n2 * 512 + 256:(n2 + 1) * 512],
                    y_sb[:, 256:512])
            else:
                nc.vector.tensor_add(y_sb, ps,
                                     bproj_bc[:, n2 * 512:(n2 + 1) * 512])
                # 4MB of f32 output vs ~60GB/s per DMA queue: all three
                # queues are needed to drain y as fast as proj produces it
                # (gpsimd only for early tiles - its queue drains late)
                order = ((nc.sync, nc.scalar, nc.gpsimd) if tb < 5
                         else (nc.sync, nc.scalar, nc.scalar))
                eng = order[(2 * tb + n2) % 3]
                eng.dma_start(y_r[:, tb, n2 * 512:(n2 + 1) * 512], y_sb)

        # ============ pipelined main loop ============
        # transposes of t-blocks 0-3 unblock qkT m=0 n2=0; qkT m=8 n2=0
        # fills the PE while x t-blocks 4-7 are still landing.
        for to in range(4):
            emit_xpose(to)
        emit_qkT(0, (0,))
        emit_qkT(NPAIR, (0,))
        for to in range(4, TO):
            emit_xpose(to)
        ps_tr.release()
        xload.release()
        ps_sc = attn_ctx.enter_context(
            tc.tile_pool(name="ps_sc", bufs=3, space="PSUM"))
        ps_pv = attn_ctx.enter_context(
            tc.tile_pool(name="ps_pv", bufs=2, space="PSUM"))
        yp = tc.alloc_tile_pool(name="ypool", bufs=6)
        # pair-1/2 weight chunks chase the x tiles on the HWDGE rings
        load_w(1, nc.sync)
        load_w(NPAIR + 1, nc.scalar)
        emit_qkT(0, (1,))
        emit_qkT(NPAIR, (1,))
        pts = emit_scores(0)
        emit_v_half(0)
        # Software pipeline: iteration j runs qkT(j+1), scores(j+1), pv(j).
        # scores(j+1) leads pv(j+1) by a whole iteration so the scalar
        # engine's exp stream (~14.7us per pair, the 2nd-busiest resource)
        # stays off the PE's critical path; qkT(j+1) keeps the PE work DENSE
        # right through the tail so the HAM clock never gates down.
        for j in range(NPAIR):
            if j + 2 < NPAIR:
                load_w(j + 2, nc.sync)
                load_w(NPAIR + j + 2, nc.scalar)
            if j + 1 < NPAIR:
                emit_qkT(j + 1)
                emit_qkT(NPAIR + j + 1)
                pts_next = emit_scores(j + 1)
            if j == NPAIR - 1:
                # The last pair's exp stream (~14.7us of scalar) has nothing
                # behind it to hide under, so interleave everything that does
                # NOT depend on pair 7 between its PV pieces (emitted BEFORE
                # the stalling matmuls - the PE queue is in-order).
                proj_open(0, 0, CO - 1)
                proj_open(0, 1, CO - 1)
                proj_open(1, 0, CO - 1)
                emit_pv(j, 0, pts)
                emit_pv(j, 1, pts, hhs=(0,))
                proj_close(0, 0)
                proj_close(0, 1)
                proj_open(1, 1, CO - 1)
                emit_pv(j, 1, pts, hhs=(1,))
            else:
                emit_pv(j, 0, pts)
                emit_pv(j, 1, pts)
            if j + 1 < NPAIR:
                pts = pts_next
            if j == 0:
                load_wv(1, nc.gpsimd)
            if j == 1:
                emit_v_half(1)
            if j in (4, 5):
                n2 = j - 4
                wproj_sb[n2] = wpp.tile([P, CO, 512], BF16,
                                        name=f"wproj{n2}", tag="wproj")
                nc.gpsimd.dma_start(wproj_sb[n2], wproj_d[n2])

        # ============ Phase P: output projection ============
        # t-blocks 0-3 need only the last pair's first q-half (cols 0-511),
        # so their final-co matmuls overlap the last PV normalization; the
        # qc=1 normalization hides under tb2/tb3's chains.
        proj_close(1, 0)
        proj_close(1, 1)
        for tb in range(2, TO):
            for n2 in range(C // 512):
                proj_open(tb, n2, CO - 1)
                proj_close(tb, n2)
        yp.release()
        attn_ctx.close()
        xT_pool.release()
        wpp.release()


_NC_CACHE = None


def _get_nc():
    global _NC_CACHE
    if _NC_CACHE is None:
        _NC_CACHE = build_nc()
    return _NC_CACHE


def kernel(x, W_qkv, b_qkv, W_proj, b_proj):
    """Full-input entry point: shards batch across 8 cores, returns [B,T,C]."""
    global LAST_RESULTS
    x = np.asarray(x, dtype=np.float32)
    W_qkv = np.asarray(W_qkv, dtype=np.float32)
    b_qkv = np.asarray(b_qkv, dtype=np.float32)
    W_proj = np.asarray(W_proj, dtype=np.float32)
    b_proj = np.asarray(b_proj, dtype=np.float32)

    scale = 1.0 / np.sqrt(HS)
    wqk = W_qkv[:, :2 * C].copy()
    wqk[:, :C] *= scale
    bqk = b_qkv[:2 * C].copy()
    bqk[:C] *= scale
    # host-side chunking for contiguous-per-partition DMA:
    # wqk4[m, p, co, r] = wqk[co*128+p, m*128+r]
    wqk4 = np.ascontiguousarray(
        wqk.reshape(CO, P, 2 * CO, P).transpose(2, 1, 0, 3)
    ).astype(ml_dtypes.bfloat16)
    wv = np.ascontiguousarray(W_qkv[:, 2 * C:])
    wv4 = wv.reshape(CO, P, 2, 512).transpose(2, 1, 0, 3).astype(
        ml_dtypes.bfloat16)
    bv = np.ascontiguousarray(b_qkv[2 * C:])
    wp4 = W_proj.reshape(CO, P, 2, 512).transpose(2, 1, 0, 3).astype(
        ml_dtypes.bfloat16)
    # mask[k, q] = 1 where q >= k (valid, causal), else 0
    mask = np.triu(np.ones((P, P), dtype=np.float32)).astype(ml_dtypes.bfloat16)
    ident = np.eye(P, dtype=np.float32).astype(ml_dtypes.bfloat16)

    xb = x.astype(ml_dtypes.bfloat16)
    common = dict(wqk=wqk4, wv=wv4, wproj=wp4, bqk=bqk, bv=bv,
                  bproj=b_proj, mask=mask, ident=ident)
    in_maps = [dict(x=np.ascontiguousarray(xb[b]), **common) for b in range(B)]

    nc = _get_nc()
    res = run_bass_kernel_spmd(nc, in_maps, core_ids=list(range(B)),
                               trace=TRACE)
    LAST_RESULTS = res
    y = np.stack([res.results[b]["y"] for b in range(B)], axis=0)
    return y
